# revision 7
# baseline (speedup 1.0000x reference)
"""AdaConv2d (per-sample masked 3x3 conv) on 8 TRN2 NeuronCores.

Strategy (data-parallel, per sharding hint):
  - 64 samples sharded 8-per-core; kernel_base/kernel_mask replicated.
  - Host ships, per sample, a [128, 114*114] bf16 buffer: BOTH partition
    halves hold the same zero-padded image (one input channel per
    partition).  The duplicate lets the two 64-row halves of the PE
    array stream independent rhs data.
  - The PE array runs in 64x64 tiling mode: 4 independent tiles
    T0/T2/T8/T10 (SBUF half x PSUM half).  Each of the 9 conv taps is a
    K=64 matmul on one tile; per output block (4 rows x 112 cols = 448
    PSUM columns) the 9 taps are split 4/5 between the two row-halves,
    accumulating into two PSUM banks (row tiles may not share a bank).
    Column halves process the even/odd block of a block-pair.  All four
    tiles stream concurrently => 4.5 effective pass-slots per block-pair
    vs 6 for the K=128 scheme.
  - Eviction per block-pair: ACT copies the second PSUM bank to SBUF
    (f32), DVE adds it to the first bank with a bf16 cast, one
    [128, 448] DMA writes the pair.
  - Per-sample kernels = kernel_base * kernel_mask[label] computed on
    device (one broadcast tensor_tensor per sample, cast to bf16).
  - Sample 0's image is shipped as four row-slabs so the first matmul
    only waits for ~1MB, not the full 3.3MB image.
"""
import numpy as np
import ml_dtypes

import concourse.bass as bass  # noqa: F401  (registers engines)
import concourse.tile as tile
from concourse import bacc, mybir
from concourse.bass_utils import run_bass_kernel_spmd

NCORES = 8
SPC = 8            # samples per core
H = W = 112
IC = OC = 64
ND = 4             # demographic groups
PW = H + 2         # padded width/height
PHW = PW * PW
RB = 4             # output rows per matmul block
N = RB * W         # 448 columns per matmul (one PSUM bank)
BLOCKS = H // RB   # 28 blocks per sample
NPAIRS = BLOCKS // 2
NT = 9             # taps
FUSE_EPOCH = 9
F32 = mybir.dt.float32
BF16 = mybir.dt.bfloat16

# sample-0 row slabs: slab k holds 34 padded rows starting at S0OFFS[k];
# pairs 0-3 use slab 0, 4-6 slab 1, 7-10 slab 2, 11-13 slab 3
S0ROWS = 34
S0COLS = S0ROWS * PW
S0OFFS = [0, 28, 56, 80]

# tap splits per (pair+blk) parity; a chain on the top SBUF half pairs
# with the complementary chain on the bottom half so every tile does
# 4+5 taps per block-pair group of two
TAPS_A4 = [(0, 0), (0, 1), (0, 2), (2, 2)]
TAPS_B5 = [(1, 0), (1, 1), (1, 2), (2, 0), (2, 1)]
TAPS_A5 = [(0, 0), (0, 1), (0, 2), (2, 0), (2, 1)]
TAPS_B4 = [(1, 0), (1, 1), (1, 2), (2, 2)]

_CACHE = {}


def _emit_pair(nc, psX, psY, w3, x3, row_off, pair):
    """All 18 tap-matmuls of one block-pair, interleaved across the four
    PE tiles so their streams overlap."""
    chains = []
    for blk in range(2):
        b = 2 * pair + blk
        r0 = 4 * b - row_off
        pc = blk * 64
        if (pair + blk) % 2 == 0:
            h0, h1 = TAPS_A4, TAPS_B5
        else:
            h0, h1 = TAPS_A5, TAPS_B4
        chains.append((psX, pc, 0, h0, r0))
        chains.append((psY, pc, 64, h1, r0))
    for i in range(5):
        for ps, pc, sp, taps, r0 in chains:
            if i >= len(taps):
                continue
            dy, dx = taps[i]
            t = 3 * dy + dx
            nc.tensor.matmul(
                ps[pc:pc + 64, :],
                w3[sp:sp + 64, t, :],
                x3[sp:sp + 64, r0 + dy:r0 + dy + RB, dx:dx + W],
                start=(i == 0), stop=(i == len(taps) - 1))


def _build():
    nc = bacc.Bacc("TRN2", target_bir_lowering=False, debug=False,
                   num_devices=NCORES)
    xs0 = nc.dram_tensor("xs0", [4, 128, S0COLS], BF16,
                         kind="ExternalInput").ap()
    xs = nc.dram_tensor("xs", [SPC - 1, 128, PHW], BF16,
                        kind="ExternalInput").ap()
    bT = nc.dram_tensor("bT", [128, NT * OC], F32,
                        kind="ExternalInput").ap()
    msel = nc.dram_tensor("msel", [128, SPC * NT], F32,
                          kind="ExternalInput").ap()
    out = nc.dram_tensor("out", [SPC, OC, H, W], BF16,
                         kind="ExternalOutput").ap()

    # [sample, pair, blk, oc, rb*w]; DMA'd from a [128, 448] SBUF tile
    # whose partition p = blk*64 + oc matches the dst iteration order
    ov = out.rearrange("b oc (r blk rb) w -> b r blk oc (rb w)",
                       blk=2, rb=RB)

    with tile.TileContext(nc) as tc:
        with (
            tc.tile_pool(name="const", bufs=1) as constp,
            tc.tile_pool(name="x0p", bufs=4) as x0p,
            tc.tile_pool(name="xp", bufs=2) as xp,
            tc.tile_pool(name="wp", bufs=2) as wp,
            tc.tile_pool(name="tmp", bufs=3) as tp,
            tc.tile_pool(name="stage", bufs=6) as stp,
            tc.tile_pool(name="psumx", bufs=4, space="PSUM") as ppx,
            tc.tile_pool(name="psumy", bufs=4, space="PSUM") as ppy,
        ):
            bT_t = constp.tile([128, NT * OC], F32, name="bT_t", tag="bT_t")
            nc.scalar.dma_start(bT_t[:], bT[:])
            msel_t = constp.tile([128, SPC * NT], F32, name="msel_t",
                                 tag="msel_t")
            nc.scalar.dma_start(msel_t[:], msel[:])
            b3 = bT_t.rearrange("p (t oc) -> p t oc", oc=OC)

            for s in range(SPC):
                wt = wp.tile([128, NT * OC], BF16, name="wt", tag="wt")
                w3 = wt.rearrange("p (t oc) -> p t oc", oc=OC)
                m3 = (msel_t[:, s * NT:(s + 1) * NT]
                      .unsqueeze(-1).broadcast_to([128, NT, OC]))
                nc.vector.tensor_tensor(w3[:], b3[:], m3,
                                        op=mybir.AluOpType.mult)

                if s == 0:
                    xts = []
                    for k in range(4):
                        xk = x0p.tile([128, S0COLS], BF16, name="x0t",
                                      tag="x0t")
                        nc.gpsimd.dma_start(xk[:], xs0[k])
                        xts.append(xk.rearrange("p (r c) -> p r c", c=PW))
                else:
                    xt = xp.tile([128, PHW], BF16, name="xt", tag="xt")
                    NCH = 8
                    for q in range(NCH):
                        qs = (PHW // NCH) * q
                        qe = PHW if q == NCH - 1 else (PHW // NCH) * (q + 1)
                        nc.gpsimd.dma_start(xt[:, qs:qe], xs[s - 1][:, qs:qe])
                    x3 = xt.rearrange("p (r c) -> p r c", c=PW)

                for pair in range(NPAIRS):
                    psX = ppx.tile([128, N], F32, name="psX", tag="psX")
                    psY = ppy.tile([128, N], F32, name="psY", tag="psY")
                    if s == 0:
                        k = (2 * pair) // 7
                        _emit_pair(nc, psX, psY, w3, xts[k], S0OFFS[k], pair)
                    else:
                        _emit_pair(nc, psX, psY, w3, x3, 0, pair)

                    tmp = tp.tile([128, N], F32, name="tmp", tag="tmp")
                    nc.scalar.copy(tmp[:], psY[:])
                    st = stp.tile([128, N], BF16, name="st", tag="st")
                    nc.vector.tensor_tensor(st[:], psX[:], tmp[:],
                                            op=mybir.AluOpType.add)
                    nc.sync.dma_start(ov[s, pair], st[:])

    nc.compile()
    return nc


def get_nc():
    if "nc" not in _CACHE:
        _CACHE["nc"] = _build()
    return _CACHE["nc"]


def make_in_maps(x, kernel_base, kernel_mask, demog_label, epoch):
    kb = np.asarray(kernel_base, dtype=np.float32)
    km = np.asarray(kernel_mask, dtype=np.float32)
    labels = np.asarray(demog_label).astype(np.int64)
    if int(np.asarray(epoch)) >= FUSE_EPOCH:
        labels = np.zeros_like(labels)

    B = labels.shape[0]
    # padded bf16 image duplicated on both partition halves
    xb = np.asarray(x, dtype=np.float32).astype(ml_dtypes.bfloat16)
    xpad = np.zeros((B, IC, PW, PW), dtype=ml_dtypes.bfloat16)
    xpad[:, :, 1:H + 1, 1:W + 1] = xb
    flat = xpad.reshape(B, IC, PHW)
    xfull = np.empty((B, 128, PHW), dtype=ml_dtypes.bfloat16)
    xfull[:, 0:IC, :] = flat
    xfull[:, IC:, :] = flat

    # bT2[p, t, oc] = kernel_base[oc, p%64, tap t], replicated halves
    kb9 = kb.reshape(OC, IC, NT)           # tap index = 3*dy + dx
    km9 = km.reshape(ND, IC, NT)
    bT2 = np.empty((128, NT, OC), dtype=np.float32)
    for t in range(NT):
        bT2[0:IC, t, :] = kb9[:, :, t].T
    bT2[IC:] = bT2[0:IC]
    bT2 = bT2.reshape(128, NT * OC)

    xr = xfull.reshape(B, 128, PW, PW)
    in_maps = []
    for c in range(NCORES):
        lab = labels[c * SPC:(c + 1) * SPC]
        msel = np.empty((128, SPC * NT), dtype=np.float32)
        for s in range(SPC):
            for t in range(NT):
                msel[0:IC, s * NT + t] = km9[lab[s], :, t]
        msel[IC:] = msel[0:IC]
        s0 = c * SPC
        xs0 = np.stack([
            np.ascontiguousarray(
                xr[s0, :, o:o + S0ROWS, :].reshape(128, S0COLS))
            for o in S0OFFS])
        in_maps.append({
            "xs0": xs0,
            "xs": np.ascontiguousarray(xfull[s0 + 1:s0 + SPC]),
            "bT": bT2,
            "msel": msel,
        })
    return in_maps


def kernel(x, kernel_base, kernel_mask, demog_label, epoch):
    nc = get_nc()
    in_maps = make_in_maps(x, kernel_base, kernel_mask, demog_label, epoch)
    res = run_bass_kernel_spmd(nc, in_maps, list(range(NCORES)))
    return np.concatenate(
        [res.results[c]["out"].astype(np.float32) for c in range(NCORES)],
        axis=0)


# revision 10
# speedup vs baseline: 2.6856x; 2.6856x over previous
"""AdaConv2d (per-sample masked 3x3 conv) on 8 TRN2 NeuronCores.

Strategy (data-parallel, per sharding hint):
  - 64 samples sharded 8-per-core; kernel_base/kernel_mask replicated.
  - Host ships, per sample, a [128, 114*114] bf16 buffer: BOTH partition
    halves hold the same zero-padded image (one input channel per
    partition).  The duplicate lets the two 64-row halves of the PE
    array stream independent rhs data.
  - The PE array runs in 64x64 tiling mode: 4 independent tiles
    T0/T2/T8/T10 (SBUF half x PSUM half).  Each of the 9 conv taps is a
    K=64 matmul on one tile; per output block (4 rows x 112 cols = 448
    PSUM columns) the 9 taps are split 4/5 between the two row-halves,
    accumulating into two PSUM banks (row tiles may not share a bank).
    Column halves process the even/odd block of a block-pair.  All four
    tiles stream concurrently => 4.5 effective pass-slots per block-pair
    vs 6 for the K=128 scheme.
  - Eviction per block-pair: ACT copies the second PSUM bank to SBUF
    (f32), DVE adds it to the first bank with a bf16 cast, one
    [128, 448] DMA writes the pair.
  - Per-sample kernels = kernel_base * kernel_mask[label] computed on
    device (one broadcast tensor_tensor per sample, cast to bf16).
  - Sample 0's image is shipped as four row-slabs so the first matmul
    only waits for ~1MB, not the full 3.3MB image.
"""
import numpy as np
import ml_dtypes

import concourse.bass as bass  # noqa: F401  (registers engines)
import concourse.tile as tile
from concourse import bacc, mybir
from concourse.bass_utils import run_bass_kernel_spmd

NCORES = 8
SPC = 8            # samples per core
H = W = 112
IC = OC = 64
ND = 4             # demographic groups
PW = H + 2         # padded width/height
PHW = PW * PW
RB = 4             # output rows per matmul block
N = RB * W         # 448 columns per matmul (one PSUM bank)
BLOCKS = H // RB   # 28 blocks per sample
NPAIRS = BLOCKS // 2
NT = 9             # taps
FUSE_EPOCH = 9
F32 = mybir.dt.float32
BF16 = mybir.dt.bfloat16

# sample-0 row slabs: slab k holds 34 padded rows starting at S0OFFS[k];
# pairs 0-3 use slab 0, 4-6 slab 1, 7-10 slab 2, 11-13 slab 3
S0ROWS = 34
S0COLS = S0ROWS * PW
S0OFFS = [0, 28, 56, 80]

# tap splits per (pair+blk) parity; a chain on the top SBUF half pairs
# with the complementary chain on the bottom half so every tile does
# 4+5 taps per block-pair group of two
TAPS_A4 = [(0, 0), (0, 1), (0, 2), (2, 2)]
TAPS_B5 = [(1, 0), (1, 1), (1, 2), (2, 0), (2, 1)]
TAPS_A5 = [(0, 0), (0, 1), (0, 2), (2, 0), (2, 1)]
TAPS_B4 = [(1, 0), (1, 1), (1, 2), (2, 2)]

_CACHE = {}


def _emit_pair(nc, psX, psY, w3, x3, row_off, pair):
    """All 18 tap-matmuls of one block-pair, interleaved across the four
    PE tiles so their streams overlap."""
    chains = []
    for blk in range(2):
        b = 2 * pair + blk
        r0 = 4 * b - row_off
        pc = blk * 64
        if (pair + blk) % 2 == 0:
            h0, h1 = TAPS_A4, TAPS_B5
        else:
            h0, h1 = TAPS_A5, TAPS_B4
        chains.append((psX, pc, 0, h0, r0))
        chains.append((psY, pc, 64, h1, r0))
    for i in range(5):
        for ps, pc, sp, taps, r0 in chains:
            if i >= len(taps):
                continue
            dy, dx = taps[i]
            t = 3 * dy + dx
            nc.tensor.matmul(
                ps[pc:pc + 64, :],
                w3[sp:sp + 64, t, :],
                x3[sp:sp + 64, r0 + dy:r0 + dy + RB, dx:dx + W],
                start=(i == 0), stop=(i == len(taps) - 1))


def _build():
    nc = bacc.Bacc("TRN2", target_bir_lowering=False, debug=False,
                   num_devices=NCORES)
    xs0 = nc.dram_tensor("xs0", [4, 128, S0COLS], BF16,
                         kind="ExternalInput").ap()
    xs = nc.dram_tensor("xs", [SPC - 1, 128, PHW], BF16,
                        kind="ExternalInput").ap()
    bT = nc.dram_tensor("bT", [128, NT * OC], F32,
                        kind="ExternalInput").ap()
    msel = nc.dram_tensor("msel", [128, SPC * NT], F32,
                          kind="ExternalInput").ap()
    # scratch layout: [sample, pair-quad q, blk*64+oc, pr*448+rb*112+w]
    # (pair = 2q+pr).  Plain 2D [128, 896] DMAs; host un-shuffles to NCHW.
    out = nc.dram_tensor("out", [SPC, NPAIRS // 2, 128, 2 * N], BF16,
                         kind="ExternalOutput").ap()

    with tile.TileContext(nc) as tc:
        with (
            tc.tile_pool(name="const", bufs=1) as constp,
            tc.tile_pool(name="x0p", bufs=4) as x0p,
            tc.tile_pool(name="xp", bufs=2) as xp,
            tc.tile_pool(name="wp", bufs=2) as wp,
            tc.tile_pool(name="tmp", bufs=3) as tp,
            tc.tile_pool(name="stage", bufs=6) as stp,
            tc.tile_pool(name="psumx", bufs=4, space="PSUM") as ppx,
            tc.tile_pool(name="psumy", bufs=4, space="PSUM") as ppy,
        ):
            bT_t = constp.tile([128, NT * OC], F32, name="bT_t", tag="bT_t")
            nc.scalar.dma_start(bT_t[:], bT[:])
            msel_t = constp.tile([128, SPC * NT], F32, name="msel_t",
                                 tag="msel_t")
            nc.scalar.dma_start(msel_t[:], msel[:])
            b3 = bT_t.rearrange("p (t oc) -> p t oc", oc=OC)

            for s in range(SPC):
                wt = wp.tile([128, NT * OC], BF16, name="wt", tag="wt")
                w3 = wt.rearrange("p (t oc) -> p t oc", oc=OC)
                m3 = (msel_t[:, s * NT:(s + 1) * NT]
                      .unsqueeze(-1).broadcast_to([128, NT, OC]))
                nc.vector.tensor_tensor(w3[:], b3[:], m3,
                                        op=mybir.AluOpType.mult)

                if s == 0:
                    xts = []
                    for k in range(4):
                        xk = x0p.tile([128, S0COLS], BF16, name="x0t",
                                      tag="x0t")
                        nc.gpsimd.dma_start(xk[:], xs0[k])
                        xts.append(xk.rearrange("p (r c) -> p r c", c=PW))
                else:
                    xt = xp.tile([128, PHW], BF16, name="xt", tag="xt")
                    NCH = 8
                    for q in range(NCH):
                        qs = (PHW // NCH) * q
                        qe = PHW if q == NCH - 1 else (PHW // NCH) * (q + 1)
                        nc.gpsimd.dma_start(xt[:, qs:qe], xs[s - 1][:, qs:qe])
                    x3 = xt.rearrange("p (r c) -> p r c", c=PW)

                for q in range(NPAIRS // 2):
                    st = stp.tile([128, 2 * N], BF16, name="st", tag="st")
                    for pr in range(2):
                        pair = 2 * q + pr
                        psX = ppx.tile([128, N], F32, name="psX", tag="psX")
                        psY = ppy.tile([128, N], F32, name="psY", tag="psY")
                        if s == 0:
                            k = (2 * pair) // 7
                            _emit_pair(nc, psX, psY, w3, xts[k],
                                       S0OFFS[k], pair)
                        else:
                            _emit_pair(nc, psX, psY, w3, x3, 0, pair)

                        tmp = tp.tile([128, N], F32, name="tmp", tag="tmp")
                        nc.scalar.copy(tmp[:], psY[:])
                        nc.vector.tensor_tensor(st[:, pr * N:(pr + 1) * N],
                                                psX[:], tmp[:],
                                                op=mybir.AluOpType.add)
                    nc.sync.dma_start(out[s, q], st[:])

    nc.compile()
    return nc


def get_nc():
    if "nc" not in _CACHE:
        _CACHE["nc"] = _build()
    return _CACHE["nc"]


def make_in_maps(x, kernel_base, kernel_mask, demog_label, epoch):
    kb = np.asarray(kernel_base, dtype=np.float32)
    km = np.asarray(kernel_mask, dtype=np.float32)
    labels = np.asarray(demog_label).astype(np.int64)
    if int(np.asarray(epoch)) >= FUSE_EPOCH:
        labels = np.zeros_like(labels)

    B = labels.shape[0]
    # padded bf16 image duplicated on both partition halves
    xb = np.asarray(x, dtype=np.float32).astype(ml_dtypes.bfloat16)
    xpad = np.zeros((B, IC, PW, PW), dtype=ml_dtypes.bfloat16)
    xpad[:, :, 1:H + 1, 1:W + 1] = xb
    flat = xpad.reshape(B, IC, PHW)
    xfull = np.empty((B, 128, PHW), dtype=ml_dtypes.bfloat16)
    xfull[:, 0:IC, :] = flat
    xfull[:, IC:, :] = flat

    # bT2[p, t, oc] = kernel_base[oc, p%64, tap t], replicated halves
    kb9 = kb.reshape(OC, IC, NT)           # tap index = 3*dy + dx
    km9 = km.reshape(ND, IC, NT)
    bT2 = np.empty((128, NT, OC), dtype=np.float32)
    for t in range(NT):
        bT2[0:IC, t, :] = kb9[:, :, t].T
    bT2[IC:] = bT2[0:IC]
    bT2 = bT2.reshape(128, NT * OC)

    xr = xfull.reshape(B, 128, PW, PW)
    in_maps = []
    for c in range(NCORES):
        lab = labels[c * SPC:(c + 1) * SPC]
        msel = np.empty((128, SPC * NT), dtype=np.float32)
        for s in range(SPC):
            for t in range(NT):
                msel[0:IC, s * NT + t] = km9[lab[s], :, t]
        msel[IC:] = msel[0:IC]
        s0 = c * SPC
        xs0 = np.stack([
            np.ascontiguousarray(
                xr[s0, :, o:o + S0ROWS, :].reshape(128, S0COLS))
            for o in S0OFFS])
        in_maps.append({
            "xs0": xs0,
            "xs": np.ascontiguousarray(xfull[s0 + 1:s0 + SPC]),
            "bT": bT2,
            "msel": msel,
        })
    return in_maps


def kernel(x, kernel_base, kernel_mask, demog_label, epoch):
    nc = get_nc()
    in_maps = make_in_maps(x, kernel_base, kernel_mask, demog_label, epoch)
    res = run_bass_kernel_spmd(nc, in_maps, list(range(NCORES)))
    outs = []
    for c in range(NCORES):
        # [s, q, blk*64+oc, pr*448+rb*112+w] -> [s, oc, h, w],
        # h = q*16 + pr*8 + blk*4 + rb
        arr = res.results[c]["out"].astype(np.float32)
        arr = arr.reshape(SPC, NPAIRS // 2, 2, OC, 2, RB, W)
        arr = arr.transpose(0, 3, 1, 4, 2, 5, 6).reshape(SPC, OC, H, W)
        outs.append(arr)
    return np.concatenate(outs, axis=0)


# revision 17
# speedup vs baseline: 2.7644x; 1.0294x over previous
"""AdaConv2d (per-sample masked 3x3 conv) on 8 TRN2 NeuronCores.

Strategy (data-parallel, per sharding hint):
  - 64 samples sharded 8-per-core; kernel_base/kernel_mask replicated.
  - Host ships, per sample, a [128, 114*114] bf16 buffer: BOTH partition
    halves hold the same zero-padded image (one input channel per
    partition).  The duplicate lets the two 64-row halves of the PE
    array stream independent rhs data.
  - The PE array runs in 64x64 tiling mode: 4 independent tiles
    T0/T2/T8/T10 (SBUF half x PSUM half).  Each of the 9 conv taps is a
    K=64 matmul on one tile; per output block (4 rows x 112 cols = 448
    PSUM columns) the 9 taps are split 4/5 between the two row-halves,
    accumulating into two PSUM banks (row tiles may not share a bank).
    Column halves process the even/odd block of a block-pair.  All four
    tiles stream concurrently => 4.5 effective pass-slots per block-pair
    vs 6 for the K=128 scheme.
  - Eviction per block-pair: ACT copies the second PSUM bank to SBUF
    (f32), DVE adds it to the first bank with a bf16 cast, one
    [128, 448] DMA writes the pair.
  - Per-sample kernels = kernel_base * kernel_mask[label] computed on
    device (one broadcast tensor_tensor per sample, cast to bf16).
  - Sample 0's image is shipped as four row-slabs so the first matmul
    only waits for ~1MB, not the full 3.3MB image.
"""
import numpy as np
import ml_dtypes

import concourse.bass as bass  # noqa: F401  (registers engines)
import concourse.tile as tile
from concourse import bacc, mybir
from concourse.bass_utils import run_bass_kernel_spmd

NCORES = 8
SPC = 8            # samples per core
H = W = 112
IC = OC = 64
ND = 4             # demographic groups
PW = H + 2         # padded width/height
PHW = PW * PW
RB = 4             # output rows per matmul block
N = RB * W         # 448 columns per matmul (one PSUM bank)
BLOCKS = H // RB   # 28 blocks per sample
NPAIRS = BLOCKS // 2
NT = 9             # taps
FUSE_EPOCH = 9
F32 = mybir.dt.float32
BF16 = mybir.dt.bfloat16

# tap splits per (pair+blk) parity; a chain on the top SBUF half pairs
# with the complementary chain on the bottom half so every tile does
# 4+5 taps per block-pair group of two
TAPS_A4 = [(0, 0), (0, 1), (0, 2), (2, 2)]
TAPS_B5 = [(1, 0), (1, 1), (1, 2), (2, 0), (2, 1)]
TAPS_A5 = [(0, 0), (0, 1), (0, 2), (2, 0), (2, 1)]
TAPS_B4 = [(1, 0), (1, 1), (1, 2), (2, 2)]

_CACHE = {}


def _emit_pair(nc, psX, psY, w3, x3, row_off, pair):
    """All 18 tap-matmuls of one block-pair, interleaved across the four
    PE tiles so their streams overlap."""
    chains = []
    for blk in range(2):
        b = 2 * pair + blk
        r0 = 4 * b - row_off
        pc = blk * 64
        if (pair + blk) % 2 == 0:
            h0, h1 = TAPS_A4, TAPS_B5
        else:
            h0, h1 = TAPS_A5, TAPS_B4
        chains.append((psX, pc, 0, h0, r0))
        chains.append((psY, pc, 64, h1, r0))
    for i in range(5):
        for ps, pc, sp, taps, r0 in chains:
            if i >= len(taps):
                continue
            dy, dx = taps[i]
            t = 3 * dy + dx
            nc.tensor.matmul(
                ps[pc:pc + 64, :],
                w3[sp:sp + 64, t, :],
                x3[sp:sp + 64, r0 + dy:r0 + dy + RB, dx:dx + W],
                start=(i == 0), stop=(i == len(taps) - 1))


def _build():
    nc = bacc.Bacc("TRN2", target_bir_lowering=False, debug=False,
                   num_devices=NCORES)
    xs = nc.dram_tensor("xs", [SPC, 128, PHW], BF16,
                        kind="ExternalInput").ap()
    bT = nc.dram_tensor("bT", [128, NT * OC], F32,
                        kind="ExternalInput").ap()
    msel = nc.dram_tensor("msel", [128, SPC * NT], F32,
                          kind="ExternalInput").ap()
    # scratch layout: [sample, pair-quad q, blk*64+oc, pr*448+rb*112+w]
    # (pair = 2q+pr).  Plain 2D [128, 896] DMAs; host un-shuffles to NCHW.
    out = nc.dram_tensor("out", [SPC, NPAIRS // 2, 128, 2 * N], BF16,
                         kind="ExternalOutput").ap()

    with tile.TileContext(nc) as tc:
        with (
            tc.tile_pool(name="const", bufs=1) as constp,
            tc.tile_pool(name="xp", bufs=2) as xp,
            tc.tile_pool(name="wp", bufs=2) as wp,
            tc.tile_pool(name="tmp", bufs=3) as tp,
            tc.tile_pool(name="stage", bufs=6) as stp,
            tc.tile_pool(name="psumx", bufs=4, space="PSUM") as ppx,
            tc.tile_pool(name="psumy", bufs=4, space="PSUM") as ppy,
        ):
            # consts on the sync queue: it idles at startup, so these tiny
            # transfers complete before the x-image flood saturates the
            # DMA engines (w-prep gates the first matmul)
            bT_t = constp.tile([128, NT * OC], F32, name="bT_t", tag="bT_t")
            nc.sync.dma_start(bT_t[:], bT[:])
            msel_t = constp.tile([128, SPC * NT], F32, name="msel_t",
                                 tag="msel_t")
            nc.sync.dma_start(msel_t[:], msel[:])
            b3 = bT_t.rearrange("p (t oc) -> p t oc", oc=OC)

            for s in range(SPC):
                wt = wp.tile([128, NT * OC], BF16, name="wt", tag="wt")
                w3 = wt.rearrange("p (t oc) -> p t oc", oc=OC)
                m3 = (msel_t[:, s * NT:(s + 1) * NT]
                      .unsqueeze(-1).broadcast_to([128, NT, OC]))
                nc.vector.tensor_tensor(w3[:], b3[:], m3,
                                        op=mybir.AluOpType.mult)

                xt = xp.tile([128, PHW], BF16, name="xt", tag="xt")
                NCH = 8
                for q in range(NCH):
                    qs = (PHW // NCH) * q
                    qe = PHW if q == NCH - 1 else (PHW // NCH) * (q + 1)
                    nc.gpsimd.dma_start(xt[:, qs:qe], xs[s][:, qs:qe])
                x3 = xt.rearrange("p (r c) -> p r c", c=PW)

                for q in range(NPAIRS // 2):
                    st = stp.tile([128, 2 * N], BF16, name="st", tag="st")
                    for pr in range(2):
                        pair = 2 * q + pr
                        psX = ppx.tile([128, N], F32, name="psX", tag="psX")
                        psY = ppy.tile([128, N], F32, name="psY", tag="psY")
                        _emit_pair(nc, psX, psY, w3, x3, 0, pair)

                        tmp = tp.tile([128, N], F32, name="tmp", tag="tmp")
                        nc.scalar.copy(tmp[:], psY[:])
                        nc.vector.tensor_tensor(st[:, pr * N:(pr + 1) * N],
                                                psX[:], tmp[:],
                                                op=mybir.AluOpType.add)
                    nc.sync.dma_start(out[s, q], st[:])

    nc.compile()
    return nc


def get_nc():
    if "nc" not in _CACHE:
        _CACHE["nc"] = _build()
    return _CACHE["nc"]


def make_in_maps(x, kernel_base, kernel_mask, demog_label, epoch):
    kb = np.asarray(kernel_base, dtype=np.float32)
    km = np.asarray(kernel_mask, dtype=np.float32)
    labels = np.asarray(demog_label).astype(np.int64)
    if int(np.asarray(epoch)) >= FUSE_EPOCH:
        labels = np.zeros_like(labels)

    B = labels.shape[0]
    # padded bf16 image duplicated on both partition halves
    xb = np.asarray(x, dtype=np.float32).astype(ml_dtypes.bfloat16)
    xpad = np.zeros((B, IC, PW, PW), dtype=ml_dtypes.bfloat16)
    xpad[:, :, 1:H + 1, 1:W + 1] = xb
    flat = xpad.reshape(B, IC, PHW)
    xfull = np.empty((B, 128, PHW), dtype=ml_dtypes.bfloat16)
    xfull[:, 0:IC, :] = flat
    xfull[:, IC:, :] = flat

    # bT2[p, t, oc] = kernel_base[oc, p%64, tap t], replicated halves
    kb9 = kb.reshape(OC, IC, NT)           # tap index = 3*dy + dx
    km9 = km.reshape(ND, IC, NT)
    bT2 = np.empty((128, NT, OC), dtype=np.float32)
    for t in range(NT):
        bT2[0:IC, t, :] = kb9[:, :, t].T
    bT2[IC:] = bT2[0:IC]
    bT2 = bT2.reshape(128, NT * OC)

    in_maps = []
    for c in range(NCORES):
        lab = labels[c * SPC:(c + 1) * SPC]
        msel = np.empty((128, SPC * NT), dtype=np.float32)
        for s in range(SPC):
            for t in range(NT):
                msel[0:IC, s * NT + t] = km9[lab[s], :, t]
        msel[IC:] = msel[0:IC]
        in_maps.append({
            "xs": np.ascontiguousarray(xfull[c * SPC:(c + 1) * SPC]),
            "bT": bT2,
            "msel": msel,
        })
    return in_maps


def kernel(x, kernel_base, kernel_mask, demog_label, epoch):
    nc = get_nc()
    in_maps = make_in_maps(x, kernel_base, kernel_mask, demog_label, epoch)
    res = run_bass_kernel_spmd(nc, in_maps, list(range(NCORES)))
    outs = []
    for c in range(NCORES):
        # [s, q, blk*64+oc, pr*448+rb*112+w] -> [s, oc, h, w],
        # h = q*16 + pr*8 + blk*4 + rb
        arr = res.results[c]["out"].astype(np.float32)
        arr = arr.reshape(SPC, NPAIRS // 2, 2, OC, 2, RB, W)
        arr = arr.transpose(0, 3, 1, 4, 2, 5, 6).reshape(SPC, OC, H, W)
        outs.append(arr)
    return np.concatenate(outs, axis=0)
